# revision 24
# baseline (speedup 1.0000x reference)
"""BottomRightPool (2D cummax) Trainium2 Bass kernel.

pool[b,c,i,j] = max(x[b,c,:i+1,:j+1])  ==  cummax over H, then over W.

Key identity: pool rows are non-decreasing along w, so
    pool[i, :] = scan_j ( state = max(state, x[i, j], pool[i-1, j]) )
because cummax_w(pool[i-1, :]) == pool[i-1, :].  tensor_tensor_scan computes
exactly  state = max(max(data0, state), data1), so ONE scan instruction per
row (data0 = x row i, data1 = pool row i-1) performs BOTH cummax passes.

Memory-bound problem => I/O in bf16 (rel tolerance 2e-2 >> bf16's 2^-9
rounding; cummax only selects values, no error accumulation; scan state is
fp32 internally). Host converts fp32<->bf16; device traffic is halved.

Layout (per core, data-parallel over the 4096 (b,c) slices):
  - 512 slices/core; tiles of [128 partitions = slices, HB*128 free = (h, w)].
  - 4 slice-chunks processed in lockstep (4-way interleave) so adjacent DVE
    scans are independent (dependent back-to-back scans stall ~1.7x).
  - The row recurrence chains across h-blocks via data1 = previous block's
    last output row.
"""

import numpy as np

N_CORES = 8
B, C, H, W = 16, 256, 128, 128
S = B * C                    # 4096 independent (b,c) slices
SPC = S // N_CORES           # 512 slices per core
CHUNK = 128                  # slices per tile (partition dim)
HB = 32                      # rows per h-block tile
LANES = 4                    # slice-chunks processed in lockstep
NEG = -3.0e38


def _np_bf16():
    import ml_dtypes

    return np.dtype(ml_dtypes.bfloat16)


def _build_nc(
    repeat=None,
    skip_scan=False,
    skip_in=False,
    skip_out=False,
    hb=None,
    bufs_in=None,
    bufs_out=None,
    split_in=1,
    split_out=1,
    q_in="sync",
    q_out="scalar",
    stagger=16,
):
    """Build the per-core Bass program. repeat=None emits the plain kernel;
    repeat=R wraps the whole workload in a hardware For_i loop (benchmarking
    only — output is just rewritten R times). skip_* flags carve out
    components for timing decomposition (results are garbage)."""
    import concourse.mybir as mybir
    import concourse.tile as tile
    from concourse import bacc

    HB = hb or globals()["HB"]
    BI = bufs_in or 2 * LANES
    BO = bufs_out or 2 * LANES + 2
    lanes = globals()["LANES"]

    nc = bacc.Bacc(None, target_bir_lowering=False)
    DT = mybir.dt.bfloat16
    xd = nc.dram_tensor("x", [SPC, H, W], DT, kind="ExternalInput")
    od = nc.dram_tensor("out", [SPC, H, W], DT, kind="ExternalOutput")
    MAX = mybir.AluOpType.max

    n_chunks = SPC // CHUNK
    assert n_chunks == LANES

    def _parts(k):
        step = HB // k
        return [(i * step, (i + 1) * step) for i in range(k)]

    parts_in, parts_out = _parts(split_in), _parts(split_out)

    if skip_in and skip_out:
        # scan-only: same 512-scan stream per iteration, tiles DMA'd once.
        with tile.TileContext(nc) as tc:
            with tc.tile_pool(name="p", bufs=1) as pool:
                As = [
                    pool.tile([CHUNK, HB * W], DT, name=f"A{i}")
                    for i in range(LANES)
                ]
                Bs = [
                    pool.tile([CHUNK, HB * W], DT, name=f"B{i}")
                    for i in range(LANES)
                ]
                for lane in range(LANES):
                    nc.sync.dma_start(
                        out=As[lane][:],
                        in_=xd[lane * CHUNK : (lane + 1) * CHUNK, 0:HB].rearrange(
                            "s h w -> s (h w)"
                        ),
                    )

                def scan_body():
                    prev = [None] * LANES
                    for hb in range(H // HB):
                        for r in range(HB):
                            row = slice(r * W, (r + 1) * W)
                            for lane in range(LANES):
                                A, Bt = As[lane], Bs[lane]
                                if r == 0 and prev[lane] is None:
                                    data1 = A[:, row]
                                elif r == 0:
                                    data1 = prev[lane]
                                else:
                                    data1 = Bt[:, (r - 1) * W : r * W]
                                nc.vector.tensor_tensor_scan(
                                    out=Bt[:, row],
                                    data0=A[:, row],
                                    data1=data1,
                                    initial=NEG,
                                    op0=MAX,
                                    op1=MAX,
                                )
                        for lane in range(LANES):
                            prev[lane] = Bs[lane][:, (HB - 1) * W : HB * W]

                if repeat is None:
                    scan_body()
                else:
                    with tc.For_i(0, repeat, 1):
                        scan_body()
                for lane in range(LANES):
                    nc.scalar.dma_start(
                        out=od[lane * CHUNK : (lane + 1) * CHUNK, 0:HB].rearrange(
                            "s h w -> s (h w)"
                        ),
                        in_=Bs[lane][:],
                    )
        nc.compile()
        return nc

    with tile.TileContext(nc) as tc:
        with tc.tile_pool(name="ina", bufs=BI) as pa, tc.tile_pool(
            name="outb", bufs=BO
        ) as pb:

            def body():
                if stagger:
                    # One continuous row stream per lane, lanes phase-shifted
                    # by `stagger` rows so block-boundary DMA bursts spread
                    # out in time instead of aligning across lanes.
                    prev = [None] * LANES
                    cur = [None] * LANES  # (A, Bt) per lane
                    for t in range(H + (LANES - 1) * stagger):
                        for lane in range(LANES):
                            rg = t - lane * stagger
                            if not (0 <= rg < H):
                                continue
                            s0 = lane * CHUNK
                            hb0, r = divmod(rg, HB)
                            h0 = hb0 * HB
                            if r == 0:
                                A = pa.tile([CHUNK, HB * W], DT, name="A")
                                Bt = pb.tile([CHUNK, HB * W], DT, name="Bt")
                                if not skip_in:
                                    for p0, p1 in parts_in:
                                        getattr(nc, q_in).dma_start(
                                            out=A[:, p0 * W : p1 * W],
                                            in_=xd[
                                                s0 : s0 + CHUNK,
                                                h0 + p0 : h0 + p1,
                                            ].rearrange("s h w -> s (h w)"),
                                        )
                                cur[lane] = (A, Bt)
                            A, Bt = cur[lane]
                            row = slice(r * W, (r + 1) * W)
                            if not skip_scan:
                                if r == 0 and prev[lane] is None:
                                    data1 = A[:, row]
                                elif r == 0:
                                    data1 = prev[lane]
                                else:
                                    data1 = Bt[:, (r - 1) * W : r * W]
                                nc.vector.tensor_tensor_scan(
                                    out=Bt[:, row],
                                    data0=A[:, row],
                                    data1=data1,
                                    initial=NEG,
                                    op0=MAX,
                                    op1=MAX,
                                )
                            if r == HB - 1:
                                prev[lane] = Bt[:, (HB - 1) * W : HB * W]
                                if not skip_out:
                                    for p0, p1 in parts_out:
                                        getattr(nc, q_out).dma_start(
                                            out=od[
                                                s0 : s0 + CHUNK,
                                                h0 + p0 : h0 + p1,
                                            ].rearrange("s h w -> s (h w)"),
                                            in_=(A if skip_scan else Bt)[
                                                :, p0 * W : p1 * W
                                            ],
                                        )
                    return
                prev = [None] * LANES
                for hb in range(H // HB):
                    h0 = hb * HB
                    tiles = []
                    for lane in range(LANES):
                        s0 = lane * CHUNK
                        A = pa.tile([CHUNK, HB * W], DT)
                        Bt = pb.tile([CHUNK, HB * W], DT)
                        if not skip_in:
                            for p0, p1 in parts_in:
                                getattr(nc, q_in).dma_start(
                                    out=A[:, p0 * W : p1 * W],
                                    in_=xd[
                                        s0 : s0 + CHUNK, h0 + p0 : h0 + p1
                                    ].rearrange("s h w -> s (h w)"),
                                )
                        tiles.append((A, Bt))
                    if not skip_scan:
                        for r in range(HB):
                            row = slice(r * W, (r + 1) * W)
                            for lane, (A, Bt) in enumerate(tiles):
                                if r == 0 and prev[lane] is None:
                                    data1 = A[:, row]
                                elif r == 0:
                                    data1 = prev[lane]
                                else:
                                    data1 = Bt[:, (r - 1) * W : r * W]
                                nc.vector.tensor_tensor_scan(
                                    out=Bt[:, row],
                                    data0=A[:, row],
                                    data1=data1,
                                    initial=NEG,
                                    op0=MAX,
                                    op1=MAX,
                                )
                    for lane, (A, Bt) in enumerate(tiles):
                        s0 = lane * CHUNK
                        prev[lane] = Bt[:, (HB - 1) * W : HB * W]
                        if not skip_out:
                            for p0, p1 in parts_out:
                                getattr(nc, q_out).dma_start(
                                    out=od[
                                        s0 : s0 + CHUNK, h0 + p0 : h0 + p1
                                    ].rearrange("s h w -> s (h w)"),
                                    in_=(A if skip_scan else Bt)[
                                        :, p0 * W : p1 * W
                                    ],
                                )

            if repeat is None:
                body()
            else:
                with tc.For_i(0, repeat, 1):
                    body()
    nc.compile()
    return nc


def make_runner(nc):
    """Compile once; return run(in_maps) plus the raw jitted callable.

    Mirrors concourse.bass2jax.run_bass_via_pjrt's multi-core path but keeps
    the jitted executable so repeated calls don't re-trace/re-compile.
    """
    import jax
    import concourse.mybir as mybir
    from jax.sharding import Mesh, NamedSharding, PartitionSpec
    from jax.experimental.shard_map import shard_map
    from concourse.bass2jax import (
        _bass_exec_p,
        install_neuronx_cc_hook,
        partition_id_tensor,
    )

    install_neuronx_cc_hook()
    assert nc.dbg_addr is None
    partition_name = nc.partition_id_tensor.name if nc.partition_id_tensor else None

    in_names, out_names, out_avals = [], [], []
    for alloc in nc.m.functions[0].allocations:
        if not isinstance(alloc, mybir.MemoryLocationSet):
            continue
        name = alloc.memorylocations[0].name
        if alloc.kind == "ExternalInput":
            if name == partition_name:
                continue
            in_names.append(name)
        elif alloc.kind == "ExternalOutput":
            out_names.append(name)
            shape = tuple(alloc.tensor_shape)
            dtype = mybir.dt.np(alloc.dtype)
            out_avals.append(jax.core.ShapedArray(shape, dtype))
    n_params = len(in_names)
    all_in_names = in_names + out_names
    if partition_name is not None:
        all_in_names = all_in_names + [partition_name]

    def _body(*args):
        operands = list(args)
        if partition_name is not None:
            operands.append(partition_id_tensor())
        outs = _bass_exec_p.bind(
            *operands,
            out_avals=tuple(out_avals),
            in_names=tuple(all_in_names),
            out_names=tuple(out_names),
            lowering_input_output_aliases=(),
            sim_require_finite=True,
            sim_require_nnan=True,
            nc=nc,
        )
        return tuple(outs)

    devices = jax.devices()[:N_CORES]
    mesh = Mesh(np.asarray(devices), ("core",))
    sharded = jax.jit(
        shard_map(
            _body,
            mesh=mesh,
            in_specs=(PartitionSpec("core"),) * (n_params + len(out_avals)),
            out_specs=(PartitionSpec("core"),) * len(out_avals),
            check_rep=False,
        ),
        keep_unused=True,
    )
    spec = NamedSharding(mesh, PartitionSpec("core"))

    def put_args(in_maps):
        """Concatenate per-core inputs (+ zero output buffers) and place them
        on the mesh once; no donation, so the same device buffers can be
        reused across calls with zero per-call H2D traffic."""
        arrs = [
            np.concatenate([np.asarray(m[name]) for m in in_maps], axis=0)
            for name in in_names
        ] + [
            np.zeros((N_CORES * a.shape[0], *a.shape[1:]), a.dtype)
            for a in out_avals
        ]
        return [jax.device_put(a, spec) for a in arrs]

    def run(in_maps):
        out_arrs = sharded(*put_args(in_maps))
        return [
            {
                name: np.asarray(out_arrs[i]).reshape(
                    N_CORES, *out_avals[i].shape
                )[c]
                for i, name in enumerate(out_names)
            }
            for c in range(N_CORES)
        ]

    return run, sharded, put_args


def make_in_maps(xf):
    """Per-core input dicts from the [S, H, W] fp32 array (bf16-converted)."""
    xb = xf.astype(_np_bf16())
    return [{"x": xb[k * SPC : (k + 1) * SPC]} for k in range(N_CORES)]


def _run(x: np.ndarray, trace: bool = False):
    """Returns (full_output, exec_time_ns_or_None)."""
    nc = _build_nc()
    run, _, _ = make_runner(nc)
    xf = np.ascontiguousarray(x, dtype=np.float32).reshape(S, H, W)
    in_maps = make_in_maps(xf)
    results = run(in_maps)
    out = np.concatenate([r["out"] for r in results], axis=0)
    return out.reshape(B, C, H, W).astype(np.float32), None


def kernel(x: np.ndarray) -> np.ndarray:
    return _run(x)[0]
